# revision 7
# baseline (speedup 1.0000x reference)
"""CrystalGraphAttention Trainium2 kernel (v2).

Data-parallel over batch: core b handles batch b (B=8, 8 cores).
Per-core algorithm (transposed layouts, [feature, node]):
  xT = x^T                                  (PE transpose)
  qT = (Wq/8)^T xT, kT = Wk^T xT            (fp32r; head pairs row-packed)
  vaug[tc] = (x Wv)[tc-chunk]               (bf16, plain [128,512] chunks)
  mT[tc]   = edge_mask^T chunk, bf16        (DMA-transpose of f32-hi-bytes;
                                             mask is 0/1 so bf16 is EXACT)
  logitsT[t,s] = kT_h^T qT_h                (row-packed pairs, one PSUM tile
                                             [128, h_even|h_odd x 512])
  ex = exp(logits * dw_t):
     ACT path: activation(Exp, scale=dw column)
     DVE path: Schraudolph exp2 bit-trick (int32 convert + bitcast)
  exm = ex * mT            (masked entries exactly 0; e_t term drops since
                            min(dw)*1e9 >> 1 for these inputs)
  po_pair[0:64] += v_he^T exm_he ; po_pair[64:128] += v_ho^T exm_ho (col-packed)
  pd[32j] += ones^T exm_h  (4-way col-packed denominators)
  oT = po / den            (recip + gpsimd partition broadcast)
  out = oT^T Wo + bo       (fp32r, bias via ones-row K=1 accumulation)
"""
import sys

if '/opt/trn_rl_repo' not in sys.path:
    sys.path.insert(0, '/opt/trn_rl_repo')

import os

import numpy as np

B, N, D = 8, 1024, 256
H, DK, DV = 8, 64, 64
NCORES = 8

# Schraudolph exp: exp(x) ~= bitcast_f32(int32(A*x + BC))
A_SCH = float(2.0 ** 23 / np.log(2.0))
B_SCH = float((127.0 - 0.044) * 2.0 ** 23)

_COMPILED = {}


def _build():
    import concourse.bass as bass
    import concourse.mybir as mybir
    import concourse.tile as tile
    from concourse import bacc
    from concourse.masks import make_identity

    f32 = mybir.dt.float32
    f32r = mybir.dt.float32r
    bf16 = mybir.dt.bfloat16
    i32 = mybir.dt.int32
    MULT = mybir.AluOpType.mult
    ADD = mybir.AluOpType.add
    EXP = mybir.ActivationFunctionType.Exp

    n_sch = int(os.environ.get("KSCH", "24"))  # of 32 a==1 pair-tiles

    nc = bacc.Bacc(None, target_bir_lowering=False)

    x_d = nc.dram_tensor("x", [N, D], f32, kind="ExternalInput")
    m_d = nc.dram_tensor("m", [N, N], bf16, kind="ExternalInput")
    dwc_d = nc.dram_tensor("dwcol", [128, 8], f32, kind="ExternalInput")
    wq_d = nc.dram_tensor("wq", [D, H * DK], f32, kind="ExternalInput")
    wk_d = nc.dram_tensor("wk", [D, H * DK], f32, kind="ExternalInput")
    wv_d = nc.dram_tensor("wv", [D, H * DV], f32, kind="ExternalInput")
    wo_d = nc.dram_tensor("wo", [H * DV, D], f32, kind="ExternalInput")
    bo_d = nc.dram_tensor("bo", [1, D], f32, kind="ExternalInput")
    out_d = nc.dram_tensor("out", [N, D], f32, kind="ExternalOutput")

    with tile.TileContext(nc) as tc:
        with tc.tile_pool(name="const", bufs=1) as cst, \
             tc.tile_pool(name="big", bufs=1) as big, \
             tc.tile_pool(name="xst", bufs=2) as xpool, \
             tc.tile_pool(name="wst", bufs=2) as wpool, \
             tc.tile_pool(name="exq", bufs=3) as expool, \
             tc.tile_pool(name="sin", bufs=3) as sinpool, \
             tc.tile_pool(name="exm", bufs=4) as exmpool, \
             tc.tile_pool(name="nrm", bufs=4) as npool, \
             tc.tile_pool(name="outp", bufs=3) as opool, \
             tc.tile_pool(name="psb", bufs=2, space="PSUM") as ps_big, \
             tc.tile_pool(name="pspo", bufs=2, space="PSUM") as ps_po, \
             tc.tile_pool(name="pspd", bufs=1, space="PSUM") as ps_pd, \
             tc.tile_pool(name="psot", bufs=1, space="PSUM") as ps_out:

            # ---- constants / tiny inputs ----
            ident = cst.tile([128, 128], f32)
            make_identity(nc, ident)

            ones_f = cst.tile([1, 128], f32)
            nc.vector.memset(ones_f, 1.0)
            ones_r = cst.tile([1, 128], f32r)
            nc.vector.tensor_copy(ones_r, ones_f)
            ones_c = cst.tile([128, 1], f32)
            nc.vector.memset(ones_c, 1.0)
            ones_bf = cst.tile([128, 1], bf16)
            nc.vector.tensor_copy(ones_bf, ones_c)

            dwc = cst.tile([128, 8], f32)
            nc.scalar.dma_start(dwc, dwc_d[:, :])
            adw = cst.tile([128, 8], f32)
            nc.vector.tensor_scalar_mul(adw, dwc, A_SCH)

            # ---- mask transposes on the sync HWDGE queue (overlap all) ----
            mT = [big.tile([128, N], bf16, name=f"mT{t8}") for t8 in range(8)]
            for t8 in range(8):
                nc.sync.dma_start_transpose(
                    mT[t8][:, :], m_d[:, t8 * 128:(t8 + 1) * 128])

            # ---- weights (scalar HWDGE queue) -> fp32r (q pre-scaled 1/8) ----
            def load_w(dram, scale):
                st = wpool.tile([128, 1024], f32, tag="wst")
                nc.scalar.dma_start(st[:, 0:512], dram[0:128, :])
                nc.scalar.dma_start(st[:, 512:1024], dram[128:256, :])
                r = big.tile([128, 1024], f32r, name=dram.name + "_r")
                if scale is None:
                    nc.vector.tensor_copy(r, st)
                else:
                    nc.vector.tensor_scalar_mul(r, st, scale)
                return r

            wq_r = load_w(wq_d, 0.125)
            wk_r = load_w(wk_d, None)
            wv_r = load_w(wv_d, None)
            wo_st = wpool.tile([128, 1024], f32, tag="wst")
            for cc in range(4):
                nc.scalar.dma_start(wo_st[:, cc * 256:(cc + 1) * 256],
                                    wo_d[cc * 128:(cc + 1) * 128, :])
            wo_r = big.tile([128, 1024], f32r)
            nc.vector.tensor_copy(wo_r, wo_st)
            bo_f = cst.tile([1, 256], f32)
            nc.scalar.dma_start(bo_f, bo_d[:, :])
            bo_r = cst.tile([1, 256], f32r)
            nc.vector.tensor_copy(bo_r, bo_f)

            # ---- xT via PE transpose ----
            xT = big.tile([128, 2 * N], f32r)  # [p=d%128, kd*1024 + n]
            for g in range(2):
                pst = ps_big.tile([128, 1024], f32, tag="ps")
                for i in range(4):
                    nch = g * 4 + i
                    xch = xpool.tile([128, D], f32, tag="xst")
                    nc.scalar.dma_start(xch, x_d[nch * 128:(nch + 1) * 128, :])
                    nc.tensor.transpose(pst[:, i * 128:(i + 1) * 128],
                                        xch[:, 0:128], ident)
                    nc.tensor.transpose(pst[:, 512 + i * 128:512 + (i + 1) * 128],
                                        xch[:, 128:256], ident)
                nc.vector.tensor_copy(
                    xT.rearrange("p (kd g n) -> p kd g n", kd=2, g=2)[:, :, g, :],
                    pst.rearrange("p (kd n) -> p kd n", kd=2))

            # ---- qT, kT (raw; dw applied at exp time) ----
            qT = big.tile([128, 4 * N], f32r)  # [dk + 64*(h%2), (h//2)*1024 + n]
            kT = big.tile([128, 4 * N], f32r)
            for c4 in range(4):
                for nt in range(2):
                    psqk = ps_big.tile([128, 1024], f32, tag="ps")
                    for kd in range(2):
                        nc.tensor.matmul(
                            psqk[:, 0:512],
                            wq_r[:, kd * 512 + c4 * 128:kd * 512 + (c4 + 1) * 128],
                            xT[:, kd * N + nt * 512:kd * N + nt * 512 + 512],
                            start=(kd == 0), stop=(kd == 1))
                    for kd in range(2):
                        nc.tensor.matmul(
                            psqk[:, 512:1024],
                            wk_r[:, kd * 512 + c4 * 128:kd * 512 + (c4 + 1) * 128],
                            xT[:, kd * N + nt * 512:kd * N + nt * 512 + 512],
                            start=(kd == 0), stop=(kd == 1))
                    off = c4 * N + nt * 512
                    nc.vector.tensor_copy(qT[:, off:off + 512], psqk[:, 0:512])
                    nc.vector.tensor_copy(kT[:, off:off + 512], psqk[:, 512:1024])

            # ---- v chunks (bf16) ----
            vaug = [big.tile([128, 512], bf16, name=f"v{t8}") for t8 in range(8)]
            for t8 in range(8):
                psv = ps_big.tile([128, 512], f32, tag="ps")
                for kd in range(2):
                    nc.tensor.matmul(
                        psv, xT[:, kd * N + t8 * 128:kd * N + (t8 + 1) * 128],
                        wv_r[:, kd * 512:(kd + 1) * 512],
                        start=(kd == 0), stop=(kd == 1))
                nc.scalar.copy(vaug[t8], psv)

            # ---- attention ----
            oT = [big.tile([128, 4 * 512], f32r, name=f"oT{st}") for st in range(2)]
            sch_used = 0
            for st in range(2):
                for half in range(2):
                    po = [ps_po.tile([128, 512], f32, tag="po",
                                     name=f"po_{st}_{half}_{a}") for a in range(2)]
                    pd = ps_pd.tile([128, 512], f32, tag="pd")
                    for t8 in range(8):
                        first, last = (t8 == 0), (t8 == 7)
                        exms = []
                        for a in range(2):
                            p_idx = half * 2 + a
                            co = p_idx * N
                            ls = ps_big.tile([128, 1024], f32, tag="ps",
                                             name=f"ls_{st}_{half}_{t8}_{a}")
                            for e in range(2):
                                nc.tensor.matmul(
                                    ls[:, e * 512:(e + 1) * 512],
                                    kT[e * 64:(e + 1) * 64,
                                       co + t8 * 128:co + (t8 + 1) * 128],
                                    qT[e * 64:(e + 1) * 64,
                                       co + st * 512:co + st * 512 + 512],
                                    start=True, stop=True)
                            mTb = mT[t8][:, st * 512:st * 512 + 512] \
                                .rearrange("p (one s) -> p one s", one=1) \
                                .broadcast_to([128, 2, 512])
                            exm_t = exmpool.tile([128, 1024], bf16, tag="exm")
                            use_sch = (a == 1) and (sch_used < n_sch)
                            if use_sch:
                                sch_used += 1
                                sint = sinpool.tile([128, 1024], i32, tag="sin")
                                nc.vector.tensor_scalar(
                                    sint, ls, adw[:, t8:t8 + 1], B_SCH, MULT, ADD)
                                nc.vector.tensor_tensor(
                                    exm_t.rearrange("p (two s) -> p two s", two=2),
                                    sint.bitcast(f32).rearrange(
                                        "p (two s) -> p two s", two=2),
                                    mTb, MULT)
                            else:
                                ex = expool.tile([128, 1024], bf16, tag="exq")
                                nc.scalar.activation(ex, ls, EXP,
                                                     scale=dwc[:, t8:t8 + 1])
                                nc.vector.tensor_tensor(
                                    exm_t.rearrange("p (two s) -> p two s", two=2),
                                    ex.rearrange("p (two s) -> p two s", two=2),
                                    mTb, MULT)
                            exms.append(exm_t)
                            for e in range(2):
                                h = 2 * p_idx + e
                                nc.tensor.matmul(
                                    po[a][e * 64:(e + 1) * 64, :],
                                    vaug[t8][:, h * 64:(h + 1) * 64],
                                    exm_t[:, e * 512:(e + 1) * 512],
                                    start=first, stop=last)
                        # 4-way col-packed denominators
                        for a in range(2):
                            for e in range(2):
                                j = 2 * a + e
                                nc.tensor.matmul(
                                    pd[32 * j:32 * j + 1, :], ones_bf,
                                    exms[a][:, e * 512:(e + 1) * 512],
                                    start=first, stop=last,
                                    tile_position=(0, 32 * j))
                    # normalize this half's 4 heads
                    den_sb = npool.tile([128, 512], f32, tag="dsb")
                    nc.vector.tensor_copy(den_sb, pd)
                    for a in range(2):
                        for e in range(2):
                            j = 2 * a + e
                            p_idx = half * 2 + a
                            p0 = 64 * e
                            drow = npool.tile([1, 512], f32, tag="drow")
                            nc.sync.dma_start(drow, den_sb[32 * j:32 * j + 1, :])
                            rr = npool.tile([1, 512], f32, tag="rr")
                            nc.vector.reciprocal_approx_fast(rr, drow)
                            rb = npool.tile([128, 512], f32, tag="rb")
                            nc.gpsimd.partition_broadcast(rb, rr)
                            nc.vector.tensor_tensor(
                                oT[st][p0:p0 + 64, p_idx * 512:(p_idx + 1) * 512],
                                po[a][p0:p0 + 64, :], rb[p0:p0 + 64, :], MULT)
                # ---- output projection for this st's 4 row-chunks ----
                for s4 in range(4):
                    sc = st * 4 + s4
                    psp = ps_out.tile([128, 256], f32, tag="psp")
                    for cc in range(4):
                        nc.tensor.matmul(
                            psp, oT[st][:, cc * 512 + s4 * 128:cc * 512 + (s4 + 1) * 128],
                            wo_r[:, cc * 256:(cc + 1) * 256],
                            start=(cc == 0), stop=False)
                    nc.tensor.matmul(psp, ones_r[0:1, :], bo_r[0:1, :],
                                     start=False, stop=True)
                    ot = opool.tile([128, 256], f32, tag="outp")
                    nc.vector.tensor_copy(ot, psp)
                    nc.scalar.dma_start(out_d[sc * 128:(sc + 1) * 128, :], ot)

    nc.compile()
    return nc


def _get_compiled():
    if 'nc' not in _COMPILED:
        _COMPILED['nc'] = _build()
    return _COMPILED['nc']


def _shard(inputs):
    import ml_dtypes
    x = np.ascontiguousarray(inputs['node_features'], dtype=np.float32)
    em = np.ascontiguousarray(inputs['edge_mask'], dtype=np.float32)
    dw = np.ascontiguousarray(inputs['distance_weights'], dtype=np.float32)
    wq = np.ascontiguousarray(inputs['Wq'], dtype=np.float32)
    wk = np.ascontiguousarray(inputs['Wk'], dtype=np.float32)
    wv = np.ascontiguousarray(inputs['Wv'], dtype=np.float32)
    wo = np.ascontiguousarray(inputs['Wo'], dtype=np.float32)
    bo = np.ascontiguousarray(inputs['bo'], dtype=np.float32).reshape(1, D)
    maps = []
    for b in range(NCORES):
        m_bf = np.ascontiguousarray(em[b, 0]).astype(ml_dtypes.bfloat16)
        maps.append({
            "x": x[b],
            "m": m_bf,
            "dwcol": np.ascontiguousarray(dw[b].reshape(8, 128).T),
            "wq": wq, "wk": wk, "wv": wv, "wo": wo, "bo": bo,
        })
    return maps


def run_sharded(inputs, **kwargs):
    from concourse.bass_utils import run_bass_kernel_spmd
    nc = _get_compiled()
    maps = _shard(inputs)
    res = run_bass_kernel_spmd(nc, maps, core_ids=list(range(NCORES)), **kwargs)
    out = np.stack([res.results[b]["out"] for b in range(NCORES)], axis=0)
    return out, res


def kernel(**inputs) -> np.ndarray:
    out, _ = run_sharded(inputs)
    return out


# revision 14
# speedup vs baseline: 1.3857x; 1.3857x over previous
"""CrystalGraphAttention Trainium2 kernel (v2).

Data-parallel over batch: core b handles batch b (B=8, 8 cores).
Per-core algorithm (transposed layouts, [feature, node]):
  xT = x^T                                  (PE transpose)
  qT = (Wq/8)^T xT, kT = Wk^T xT            (fp32r; head pairs row-packed)
  vaug[tc] = (x Wv)[tc-chunk]               (bf16, plain [128,512] chunks)
  mT[tc]   = edge_mask^T chunk, bf16        (DMA-transpose of f32-hi-bytes;
                                             mask is 0/1 so bf16 is EXACT)
  logitsT[t,s] = kT_h^T qT_h                (row-packed pairs, one PSUM tile
                                             [128, h_even|h_odd x 512])
  ex = exp(logits * dw_t):
     ACT path: activation(Exp, scale=dw column)
     DVE path: Schraudolph exp2 bit-trick (int32 convert + bitcast)
  exm = ex * mT            (masked entries exactly 0; e_t term drops since
                            min(dw)*1e9 >> 1 for these inputs)
  po_pair[0:64] += v_he^T exm_he ; po_pair[64:128] += v_ho^T exm_ho (col-packed)
  pd[32j] += ones^T exm_h  (4-way col-packed denominators)
  oT = po / den            (recip + gpsimd partition broadcast)
  out = oT^T Wo + bo       (fp32r, bias via ones-row K=1 accumulation)
"""
import sys

if '/opt/trn_rl_repo' not in sys.path:
    sys.path.insert(0, '/opt/trn_rl_repo')

import os

import numpy as np

B, N, D = 8, 1024, 256
H, DK, DV = 8, 64, 64
NCORES = 8

# Schraudolph exp: exp(x) ~= bitcast_f32(int32(A*x + BC))
A_SCH = float(2.0 ** 23 / np.log(2.0))
B_SCH = float((127.0 - 0.044) * 2.0 ** 23)

_COMPILED = {}


def _build():
    import concourse.bass as bass
    import concourse.mybir as mybir
    import concourse.tile as tile
    from concourse import bacc
    from concourse.masks import make_identity

    f32 = mybir.dt.float32
    f32r = mybir.dt.float32r
    bf16 = mybir.dt.bfloat16
    i32 = mybir.dt.int32
    MULT = mybir.AluOpType.mult
    ADD = mybir.AluOpType.add
    EXP = mybir.ActivationFunctionType.Exp

    n_sch = int(os.environ.get("KSCH", "0"))  # of 32 a==1 pair-tiles

    nc = bacc.Bacc(None, target_bir_lowering=False)

    x_d = nc.dram_tensor("x", [N, D], f32, kind="ExternalInput")
    m_d = nc.dram_tensor("m", [N, N], bf16, kind="ExternalInput")
    dwc_d = nc.dram_tensor("dwcol", [128, 8], f32, kind="ExternalInput")
    wq_d = nc.dram_tensor("wq", [D, H * DK], f32, kind="ExternalInput")
    wk_d = nc.dram_tensor("wk", [D, H * DK], f32, kind="ExternalInput")
    wv_d = nc.dram_tensor("wv", [D, H * DV], f32, kind="ExternalInput")
    wo_d = nc.dram_tensor("wo", [H * DV, D], f32, kind="ExternalInput")
    bo_d = nc.dram_tensor("bo", [1, D], f32, kind="ExternalInput")
    out_d = nc.dram_tensor("out", [N, D], f32, kind="ExternalOutput")

    with tile.TileContext(nc) as tc:
        with tc.tile_pool(name="const", bufs=1) as cst, \
             tc.tile_pool(name="big", bufs=1) as big, \
             tc.tile_pool(name="xst", bufs=2) as xpool, \
             tc.tile_pool(name="wst", bufs=2) as wpool, \
             tc.tile_pool(name="exq", bufs=3) as expool, \
             tc.tile_pool(name="sin", bufs=3) as sinpool, \
             tc.tile_pool(name="exm", bufs=4) as exmpool, \
             tc.tile_pool(name="nrm", bufs=4) as npool, \
             tc.tile_pool(name="outp", bufs=3) as opool, \
             tc.tile_pool(name="psb", bufs=2, space="PSUM") as ps_big, \
             tc.tile_pool(name="pspo", bufs=2, space="PSUM") as ps_po, \
             tc.tile_pool(name="pspd", bufs=1, space="PSUM") as ps_pd, \
             tc.tile_pool(name="psot", bufs=1, space="PSUM") as ps_out:

            # ---- constants / tiny inputs ----
            ident = cst.tile([128, 128], f32)
            make_identity(nc, ident)

            ones_f = cst.tile([1, 128], f32)
            nc.vector.memset(ones_f, 1.0)
            ones_r = cst.tile([1, 128], f32r)
            nc.vector.tensor_copy(ones_r, ones_f)
            ones_c = cst.tile([128, 1], f32)
            nc.vector.memset(ones_c, 1.0)
            ones_bf = cst.tile([128, 1], bf16)
            nc.vector.tensor_copy(ones_bf, ones_c)

            dwc = cst.tile([128, 8], f32)
            nc.scalar.dma_start(dwc, dwc_d[:, :])
            adw = cst.tile([128, 8], f32)
            nc.vector.tensor_scalar_mul(adw, dwc, A_SCH)

            # ---- bulk input DMAs: x first (scalar queue), mask on sync ----
            xsb = big.tile([128, 8, D], f32, name="xsb")
            nc.scalar.dma_start(xsb, x_d.rearrange("(nch p) j -> p nch j", p=128))

            mT = [big.tile([128, N], bf16, name=f"mT{t8}") for t8 in range(8)]
            for t8 in range(8):
                nc.sync.dma_start(mT[t8], m_d[t8 * 128:(t8 + 1) * 128, :])

            # ---- weights (scalar HWDGE queue) -> fp32r (q pre-scaled 1/8) ----
            def load_w(dram, scale):
                st = wpool.tile([128, 1024], f32, tag="wst")
                nc.scalar.dma_start(
                    st.rearrange("p (kd c) -> p kd c", kd=2),
                    dram.rearrange("(kd p) c -> p kd c", p=128))
                r = big.tile([128, 1024], f32r, name=dram.name + "_r")
                if scale is None:
                    nc.vector.tensor_copy(r, st)
                else:
                    nc.vector.tensor_scalar_mul(r, st, scale)
                return r

            wq_r = load_w(wq_d, 0.125)
            wk_r = load_w(wk_d, None)
            wv_r = load_w(wv_d, None)
            wo_st = wpool.tile([128, 1024], f32, tag="wst")
            nc.scalar.dma_start(
                wo_st.rearrange("p (cc c) -> p cc c", cc=4),
                wo_d.rearrange("(cc p) c -> p cc c", p=128))
            wo_r = big.tile([128, 1024], f32r)
            nc.vector.tensor_copy(wo_r, wo_st)
            bo_f = cst.tile([1, 256], f32)
            nc.scalar.dma_start(bo_f, bo_d[:, :])
            bo_r = cst.tile([1, 256], f32r)
            nc.vector.tensor_copy(bo_r, bo_f)

            # ---- xT via PE transpose ----
            xT = big.tile([128, 2 * N], f32r)  # [p=d%128, kd*1024 + n]
            for g in range(2):
                pst = ps_big.tile([128, 1024], f32, tag="ps")
                for i in range(4):
                    nch = g * 4 + i
                    nc.tensor.transpose(pst[:, i * 128:(i + 1) * 128],
                                        xsb[:, nch, 0:128], ident)
                    nc.tensor.transpose(pst[:, 512 + i * 128:512 + (i + 1) * 128],
                                        xsb[:, nch, 128:256], ident)
                nc.vector.tensor_copy(
                    xT.rearrange("p (kd g n) -> p kd g n", kd=2, g=2)[:, :, g, :],
                    pst.rearrange("p (kd n) -> p kd n", kd=2))

            # ---- qT, kT (raw; dw applied at exp time) ----
            qT = big.tile([128, 4 * N], f32r)  # [dk + 64*(h%2), (h//2)*1024 + n]
            kT = big.tile([128, 4 * N], f32r)
            for c4 in range(4):
                for nt in range(2):
                    psqk = ps_big.tile([128, 1024], f32, tag="ps")
                    for kd in range(2):
                        nc.tensor.matmul(
                            psqk[:, 0:512],
                            wq_r[:, kd * 512 + c4 * 128:kd * 512 + (c4 + 1) * 128],
                            xT[:, kd * N + nt * 512:kd * N + nt * 512 + 512],
                            start=(kd == 0), stop=(kd == 1))
                    for kd in range(2):
                        nc.tensor.matmul(
                            psqk[:, 512:1024],
                            wk_r[:, kd * 512 + c4 * 128:kd * 512 + (c4 + 1) * 128],
                            xT[:, kd * N + nt * 512:kd * N + nt * 512 + 512],
                            start=(kd == 0), stop=(kd == 1))
                    off = c4 * N + nt * 512
                    nc.scalar.copy(qT[:, off:off + 512], psqk[:, 0:512])
                    nc.vector.tensor_copy(kT[:, off:off + 512], psqk[:, 512:1024])

            # ---- v chunks (bf16) ----
            vaug = [big.tile([128, 512], bf16, name=f"v{t8}") for t8 in range(8)]
            for t8 in range(8):
                psv = ps_big.tile([128, 512], f32, tag="ps")
                for kd in range(2):
                    nc.tensor.matmul(
                        psv, xT[:, kd * N + t8 * 128:kd * N + (t8 + 1) * 128],
                        wv_r[:, kd * 512:(kd + 1) * 512],
                        start=(kd == 0), stop=(kd == 1))
                nc.vector.tensor_copy(vaug[t8], psv)

            # ---- attention ----
            oT = [big.tile([128, 4 * 512], f32r, name=f"oT{st}") for st in range(2)]
            sch_used = 0
            for st in range(2):
                for half in range(2):
                    po = [ps_po.tile([128, 512], f32, tag="po",
                                     name=f"po_{st}_{half}_{a}") for a in range(2)]
                    pd = ps_pd.tile([128, 512], f32, tag="pd")
                    for t8 in range(8):
                        first, last = (t8 == 0), (t8 == 7)
                        exms = []
                        for a in range(2):
                            p_idx = half * 2 + a
                            co = p_idx * N
                            ls = ps_big.tile([128, 1024], f32, tag="ps",
                                             name=f"ls_{st}_{half}_{t8}_{a}")
                            for e in range(2):
                                nc.tensor.matmul(
                                    ls[:, e * 512:(e + 1) * 512],
                                    kT[e * 64:(e + 1) * 64,
                                       co + t8 * 128:co + (t8 + 1) * 128],
                                    qT[e * 64:(e + 1) * 64,
                                       co + st * 512:co + st * 512 + 512],
                                    start=True, stop=True)
                            mTb = mT[t8][:, st * 512:st * 512 + 512] \
                                .rearrange("p (one s) -> p one s", one=1) \
                                .broadcast_to([128, 2, 512])
                            exm_t = exmpool.tile([128, 1024], bf16, tag="exm")
                            use_sch = (a == 1) and (sch_used < n_sch)
                            if use_sch:
                                sch_used += 1
                                sint = sinpool.tile([128, 1024], i32, tag="sin")
                                nc.vector.tensor_scalar(
                                    sint, ls, adw[:, t8:t8 + 1], B_SCH, MULT, ADD)
                                nc.vector.tensor_tensor(
                                    exm_t.rearrange("p (two s) -> p two s", two=2),
                                    sint.bitcast(f32).rearrange(
                                        "p (two s) -> p two s", two=2),
                                    mTb, MULT)
                            else:
                                ex = expool.tile([128, 1024], bf16, tag="exq")
                                nc.scalar.activation(ex, ls, EXP,
                                                     scale=dwc[:, t8:t8 + 1])
                                nc.vector.tensor_tensor(
                                    exm_t.rearrange("p (two s) -> p two s", two=2),
                                    ex.rearrange("p (two s) -> p two s", two=2),
                                    mTb, MULT)
                            exms.append(exm_t)
                            for e in range(2):
                                h = 2 * p_idx + e
                                nc.tensor.matmul(
                                    po[a][e * 64:(e + 1) * 64, :],
                                    vaug[t8][:, h * 64:(h + 1) * 64],
                                    exm_t[:, e * 512:(e + 1) * 512],
                                    start=first, stop=last)
                        # 4-way col-packed denominators
                        for a in range(2):
                            for e in range(2):
                                j = 2 * a + e
                                nc.tensor.matmul(
                                    pd[32 * j:32 * j + 1, :], ones_bf,
                                    exms[a][:, e * 512:(e + 1) * 512],
                                    start=first, stop=last,
                                    tile_position=(0, 32 * j))
                    # normalize this half's 4 heads (one folded reciprocal)
                    rden = npool.tile([128, 512], f32, tag="dsb")
                    nc.vector.reciprocal_approx_fast(rden, pd)
                    for a in range(2):
                        for e in range(2):
                            j = 2 * a + e
                            p_idx = half * 2 + a
                            p0 = 64 * e
                            rr = npool.tile([1, 512], f32, tag="rr")
                            nc.sync.dma_start(rr, rden[32 * j:32 * j + 1, :])
                            rb = npool.tile([128, 512], f32, tag="rb")
                            nc.gpsimd.partition_broadcast(rb, rr)
                            nc.vector.tensor_tensor(
                                oT[st][p0:p0 + 64, p_idx * 512:(p_idx + 1) * 512],
                                po[a][p0:p0 + 64, :], rb[p0:p0 + 64, :], MULT)
                # ---- output projection for this st's 4 row-chunks ----
                for s4 in range(4):
                    sc = st * 4 + s4
                    psp = ps_out.tile([128, 256], f32, tag="psp")
                    for cc in range(4):
                        nc.tensor.matmul(
                            psp, oT[st][:, cc * 512 + s4 * 128:cc * 512 + (s4 + 1) * 128],
                            wo_r[:, cc * 256:(cc + 1) * 256],
                            start=(cc == 0), stop=False)
                    nc.tensor.matmul(psp, ones_r[0:1, :], bo_r[0:1, :],
                                     start=False, stop=True)
                    ot = opool.tile([128, 256], f32, tag="outp")
                    nc.vector.tensor_copy(ot, psp)
                    nc.scalar.dma_start(out_d[sc * 128:(sc + 1) * 128, :], ot)

    nc.compile()
    return nc


def _get_compiled():
    if 'nc' not in _COMPILED:
        _COMPILED['nc'] = _build()
    return _COMPILED['nc']


def _shard(inputs):
    import ml_dtypes
    x = np.ascontiguousarray(inputs['node_features'], dtype=np.float32)
    em = np.ascontiguousarray(inputs['edge_mask'], dtype=np.float32)
    dw = np.ascontiguousarray(inputs['distance_weights'], dtype=np.float32)
    wq = np.ascontiguousarray(inputs['Wq'], dtype=np.float32)
    wk = np.ascontiguousarray(inputs['Wk'], dtype=np.float32)
    wv = np.ascontiguousarray(inputs['Wv'], dtype=np.float32)
    wo = np.ascontiguousarray(inputs['Wo'], dtype=np.float32)
    bo = np.ascontiguousarray(inputs['bo'], dtype=np.float32).reshape(1, D)
    maps = []
    for b in range(NCORES):
        m_bf = np.ascontiguousarray(em[b, 0].T).astype(ml_dtypes.bfloat16)
        maps.append({
            "x": x[b],
            "m": m_bf,
            "dwcol": np.ascontiguousarray(dw[b].reshape(8, 128).T),
            "wq": wq, "wk": wk, "wv": wv, "wo": wo, "bo": bo,
        })
    return maps


def run_sharded(inputs, **kwargs):
    from concourse.bass_utils import run_bass_kernel_spmd
    nc = _get_compiled()
    maps = _shard(inputs)
    res = run_bass_kernel_spmd(nc, maps, core_ids=list(range(NCORES)), **kwargs)
    out = np.stack([res.results[b]["out"] for b in range(NCORES)], axis=0)
    return out, res


def kernel(**inputs) -> np.ndarray:
    out, _ = run_sharded(inputs)
    return out
